# revision 41
# baseline (speedup 1.0000x reference)
# Trainium2 Bass kernel for nn_Adapter_22342419874228.
#
# Per row r of logits [B=16384, C=1000]:
#   prob = softmax(logits); order = argsort(-prob); sp = prob[order]
#   diffs = [sqrt(sp[j]-sp[j+1]) ... , 1]
#   raw = diffs * [sigmoid((prob@W.T+b)[:999]), (prob@W.T+b)[999]]
#   rc = reverse-cumsum(raw); fitted[r, order[j]] = rc[j]
#
# Device algorithm (data-parallel over 8 cores, 2048 rows each, 16 tiles of
# 128 rows; two 8-tile sorts, epilogues pipelined in 4-tile subgroups):
#   e = exp(logits) (logits are in [-5.5, 5.5], no max-subtraction needed);
#   Z = rowsum(e).  Sort key = float32 bits of e with the low 10 mantissa
#   bits replaced by the column index: a positive float whose fp32 min/max
#   order equals the (truncated-e, idx) lexicographic order, so a
#   single-plane sort carries values and indices together.
#   The sort is a 55-substage Batcher odd-even merge network (24063
#   comparators/row), run in place on the vector engine with one
#   instruction per substage: a custom dual-write compare-exchange
#   program installed over the stock COPY opcode's 2x_2p perf slot
#   executes one compare-exchange per cycle (2x the stock
#   tensor_tensor max+min pair) - see the block comment below.
#   fc is computed as e_ext @ WT_ext where e_ext has an extra column = Z
#   and WT_ext an extra row = b, so fc/Z = prob@W.T + b without
#   normalizing e.  sigmoid(x) = 0.5*(1+tanh(x/2)) keeps the scalar engine
#   on one activation table; sqrt(gap * 0.25/Z) folds the 0.5/sqrt(Z)
#   diff normalization into the activation scale.
#   rc comes from an inclusive cumsum (its tail is the row total T):
#   rc[j] = T - C[j-1], evaluated on the scalar engine as
#   Identity(C * -1 + T).  fitted is produced by a per-partition
#   local_scatter (GPSIMD) of rc at the sorted indices.
import numpy as np
import ml_dtypes

import concourse.bass as bass
import concourse.tile as tile
from concourse import bacc
from concourse import mybir, library_config
from concourse import dve_ops
from concourse.bass_utils import run_bass_kernel_spmd
from concourse.dve_spec import Spec, Src0, maxx
from concourse.dve_uop import (
    DveOpSpec, UopConfig, AluOp, AluInp, InpSel, OutSel, OutPath,
    Trigger, DelayInp,
)
from concourse.masks import make_identity

BATCH = 16384
C = 1000
NCORES = 8
ROWS = BATCH // NCORES        # 2048 rows per core
P = 128                       # partition tile
NTILES = ROWS // P            # 16
NG = 4                        # tiles per epilogue subgroup
NSG = 8                       # tiles per fused sort
N = 1024                      # padded sort width
KEXT = 1008                   # contraction length: 1000 + Z col + 7 zero pad
KCH = 126                     # 8 chunks of 126 = 1008

AF = mybir.ActivationFunctionType
OP = mybir.AluOpType

# ---------------------------------------------------------------------------
# Dual-write compare-exchange via a hijacked stock COPY opcode.
#
# The per-NEFF DVE table generator can repoint any opcode row at custom uOps.
# We replace COPY's (0x46) program: the 2x_2p perf slot becomes a pairwise
# minmax.  In 2x_2p mode the engine splits the src AND dst AP enumerations
# at the midpoint across its two read / two write ports:
#   SRC_0[t] = src AP position t            (first half  = lows)
#   SRC_1[t] = src AP position n+t          (second half = partners)
#   WR0[t] -> dst position t      = max(SRC_0[t], SRC_1[t])
#   WR1[t] -> dst position n+t    = min(...)
# at 1 pair/cycle - exactly one bitonic compare-exchange substage in a
# single instruction at 2x the stock tensor_tensor rate (hardware-verified).
# The 1x fallback slot writes +inf poison so an unexpected mode fallback
# fails loudly in the correctness check rather than silently mis-sorting.

MINMAX_NAME = "MINMAX_UP_COPY_ANT"


def _uop_poison_1x() -> UopConfig:
    u = UopConfig()
    u.enable_input(InpSel.SRC_0, 0)
    u.require_inp0 = 1
    u.trigger = (Trigger.SRC_TENSOR_DONE, Trigger.NONE, Trigger.NONE)
    dp = u.datapath_config
    dp[0].enable_alu(AluOp.BYPASS, AluInp.PREV_DELAY_0, AluInp.PREV_DELAY_0)
    dp[0].enable_delay_from_src(DelayInp.PREV_ALU_OUT, 0)
    u.enable_input(InpSel.POS_INF, 1)
    for k in range(1, 8):
        dp[k].pass_through_alu()
    u.enable_output(OutSel.ALU_OUT, OutPath.WR0_LO)
    return u


def _uop_copy_2x1p() -> UopConfig:
    # clone of stock slot 33 (2x_1p packed 16-bit copy); unreachable for
    # fp32 data but the perf layout requires the slot.
    u = UopConfig()
    u.enable_input(InpSel.SRC_0, 0)
    u.enable_input(InpSel.SRC_0_HI, 1)
    u.require_inp0 = 1
    u.trigger = (Trigger.SRC_TENSOR_DONE, Trigger.NONE, Trigger.NONE)
    for k in range(8):
        u.datapath_config[k].pass_through_alu().pass_through_delay(0)
    u.enable_output(OutSel.ALU_OUT, OutPath.WR0_LO)
    u.enable_output(OutSel.DELAY_0, OutPath.WR0_HI)
    return u


def _uop_minmax_2x2p() -> UopConfig:
    u = UopConfig()
    u.enable_input(InpSel.SRC_0, 0)   # a -> stage0 PREV_ALU_OUT
    u.enable_input(InpSel.SRC_1, 2)   # b -> stage0 PREV_DELAY_1
    u.require_inp0 = 1
    u.require_inp1 = 1
    u.trigger = (Trigger.SRC_TENSOR_DONE, Trigger.NONE, Trigger.NONE)
    dp = u.datapath_config
    dp[0].enable_alu(AluOp.MAX, AluInp.PREV_ALU_OUT, AluInp.PREV_DELAY_1)
    dp[0].enable_delay_from_src(DelayInp.PREV_ALU_OUT, 0)  # capture a
    dp[0].pass_through_delay(1)                            # carry b
    dp[1].enable_alu(AluOp.MIN, AluInp.PREV_DELAY_0, AluInp.PREV_DELAY_1)
    dp[1].enable_delay_from_src(DelayInp.PREV_ALU_OUT, 2)  # capture max
    for k in range(2, 8):
        dp[k].pass_through_alu().pass_through_delay(2)
    u.enable_output(OutSel.DELAY_2, OutPath.WR0_LO)  # max -> lows half
    u.enable_output(OutSel.ALU_OUT, OutPath.WR1_LO)  # min -> partners half
    return u


class _RawDveOp:
    """DveOp-compatible wrapper around a hand-built DveOpSpec."""

    def __init__(self, name, dvespec):
        self.name = name
        self.subdim = False
        self.spec = Spec(body=maxx(Src0, Src0), reference=None)
        self._compiled = dvespec

    def compile(self, ver):
        return self._compiled


def _register_minmax():
    if MINMAX_NAME in dve_ops._SUB_OPCODE_FOR_NAME:
        return
    spec = DveOpSpec(
        name=MINMAX_NAME,
        uops=[_uop_poison_1x()],
        uops_2x=[_uop_copy_2x1p()],
        uops_2x_2p=[_uop_minmax_2x2p()],
        uops_4x=None,
        opcode=0x46,   # stock COPY opcode row
        rd1_en=False,
    )
    spec.validate("v3")
    op = _RawDveOp(MINMAX_NAME, spec)
    dve_ops.OPS.append(op)
    dve_ops._SUB_OPCODE_FOR_NAME[MINMAX_NAME] = 0x46
    dve_ops.CUSTOM_DVE_SPECS[MINMAX_NAME] = op.spec


_register_minmax()


def _int_bitop(eng, out, in0, imm, imm_dtype, op0, in1=None, op1=None,
               imm2=None):
    """Emit a TensorScalarPtr with integer-typed immediates (the BIR
    verifier requires bitvec-op immediates to match the src/dst dtype).
    in1 gives a scalar_tensor_tensor second operand; imm2 a second
    immediate for a dual tensor-scalar op."""
    ins = [eng.lower_ap(in0), mybir.ImmediateValue(dtype=imm_dtype, value=imm)]
    kw = {}
    if in1 is not None:
        ins.append(eng.lower_ap(in1))
        kw = dict(is_scalar_tensor_tensor=True)
    elif imm2 is not None:
        ins.append(mybir.ImmediateValue(dtype=imm_dtype, value=imm2))
    return eng.add_instruction(mybir.InstTensorScalarPtr(
        name=eng.bass.get_next_instruction_name(),
        op0=op0, op1=(op1 if op1 is not None else OP.bypass),
        ins=ins, outs=[eng.lower_ap(out)], **kw))


def _clean(dims):
    """Drop count-1 dims and merge [s*c, n],[s, c] pairs so the lowered AP
    has no degenerate dimensions (degenerate dims knock the DVE address
    generator off its fast path)."""
    dims = [d for d in dims if d[1] != 1]
    out = []
    for d in dims:
        if out and out[-1][0] == d[0] * d[1]:
            out[-1] = [d[0], d[1] * out[-1][1]]
        else:
            out.append(list(d))
    return out if out else [[1, 1]]


def _ce_pair(eng, src, dst, lows_off, lows_dims, part_off, part_dims):
    """One compare-exchange substage across all NG tile lanes: max of
    (lows, partners) to the low positions of dst, min to the partners."""
    mk = lambda t, off, dims: bass.AP(
        tensor=t.tensor, offset=t.offset + off,
        ap=[t.ap[0]] + _clean([[N, NG]] + dims)
    )
    lows_src = mk(src, lows_off, lows_dims)
    part_src = mk(src, part_off, part_dims)
    lows_dst = mk(dst, lows_off, lows_dims)
    part_dst = mk(dst, part_off, part_dims)
    eng.tensor_tensor(out=lows_dst, in0=lows_src, in1=part_src, op=OP.max)
    eng.tensor_tensor(out=part_dst, in0=lows_src, in1=part_src, op=OP.min)


def _ce_fused(nc, src, dst, j):
    """One j-substage as a single dual-write compare-exchange instruction
    (the hijacked COPY): src/dst AP = [[j, 2], [2j, G], [1, j]] whose halves
    are exactly (lows, partners); the 2x_2p minmax program writes max to the
    lows half and min to the partners half at 1 pair/cycle."""
    G = NG * N // (2 * j)
    dims = _clean([[j, 2], [2 * j, G], [1, j]])
    mk = lambda t: bass.AP(tensor=t.tensor, offset=t.offset,
                           ap=[t.ap[0]] + dims)
    nc.vector.tensor_copy(mk(dst), mk(src))


def _emit_sort(nc, bufA, base_tile=0, ntiles=NSG):
    """Batcher odd-even merge sort, descending, in place on `ntiles` tiles
    of bufA starting at `base_tile`.  24063 comparators vs bitonic's 28160,
    every comparator is max-to-low, and each substage touches each position
    at most once, so every substage is one in-place fused dual-write CE
    instruction.  Returns the buffer holding the sorted result."""
    base = base_tile * N
    p = 1
    while p < N:
        k = p
        while k >= 1:
            if k == p:
                off = 0
                dims = [[k, 2], [2 * p, ntiles * N // (2 * p)], [1, p]]
            else:
                off = k
                dims = [[k, 2], [2 * p, ntiles * N // (2 * p)],
                        [2 * k, p // k - 1], [1, k]]
            dims = _clean_keep2(dims)
            ap = bass.AP(tensor=bufA.tensor, offset=bufA.offset + base + off,
                         ap=[bufA.ap[0]] + dims)
            nc.vector.tensor_copy(ap, ap)
            k //= 2
        p *= 2
    return bufA


def _clean_keep2(dims):
    """_clean, but never merge into the outermost [k, 2] pair dim (its
    count-2 halves are what the engine splits across its ports)."""
    head, rest = dims[:1], _clean(dims[1:])
    return head + rest


def build_nc():
    nc = bacc.Bacc(None, target_bir_lowering=False)
    l_in = nc.dram_tensor("logits", [ROWS, C], mybir.dt.float32, kind="ExternalInput")
    wt_in = nc.dram_tensor("wt", [KEXT, C], mybir.dt.bfloat16, kind="ExternalInput")
    ltri_in = nc.dram_tensor("ltri", [KCH, 8, C], mybir.dt.bfloat16,
                             kind="ExternalInput")
    out_d = nc.dram_tensor("out", [ROWS, C], mybir.dt.bfloat16, kind="ExternalOutput")

    with tile.TileContext(nc) as tc:
        with (
            tc.tile_pool(name="singles", bufs=1) as singles,
            tc.tile_pool(name="inp", bufs=5) as inp,
            tc.tile_pool(name="smallp", bufs=40) as smallp,
            tc.tile_pool(name="front", bufs=4) as front,
            tc.tile_pool(name="ebtp", bufs=2) as ebtp,
            tc.tile_pool(name="grp", bufs=1) as grp,
            tc.tile_pool(name="outp", bufs=2) as outp,
            tc.tile_pool(name="psumF", bufs=1, space=bass.MemorySpace.PSUM) as psumF,
            tc.tile_pool(name="psumE", bufs=1, space=bass.MemorySpace.PSUM) as psumE,
        ):
            iota = singles.tile([P, N], mybir.dt.int32)
            ident = singles.tile([P, P], mybir.dt.float32)
            wt = singles.tile([KCH, 8, C], mybir.dt.bfloat16)

            # L[k, j] = 1 if k >= j (k-chunked):  rc = raw @ L gives the
            # reverse cumulative sum on the PE instead of a slow DVE scan.
            L = singles.tile([KCH, 8, C], mybir.dt.bfloat16)
            nc.gpsimd.iota(iota[:], [[1, N]], channel_multiplier=0)
            make_identity(nc, ident[:])

            def deferred_init():
                # emitted after the first group's input DMAs so 2MB of
                # weight traffic doesn't queue ahead of the logits the
                # first sort is waiting on; consumers (matmul, epi2,
                # scatter) only run once the first sort is in flight.
                # L comes precomputed from the host: building it on GPSIMD
                # (8 affine_selects, ~20us) steals the SBUF port the DVE's
                # 2-port sort instructions need.
                for k in range(8):
                    nc.sync.dma_start(L[:, k, :],
                                      ltri_in[:, k, :])
                for k in range(8):
                    nc.sync.dma_start(wt[:, k, :],
                                      wt_in[k * KCH:(k + 1) * KCH, :])
                nc.gpsimd.load_library(library_config.local_scatter)

            # sort buffer spans NSG=8 tiles (two 4-tile epi subgroups per
            # sort), halving the per-substage instruction fixed cost
            bufA = grp.tile([P, NSG, N], mybir.dt.float32)
            bufB = None   # OEMS sorts in place; no ping-pong buffer
            # per-subgroup diffs/idx buffers so subgroup 1's epi1 does not
            # serialize behind subgroup 0's epi2 consumers
            dsgs = [grp.tile([P, NG, C], mybir.dt.float32, name="dsgA"),
                    grp.tile([P, NG, C], mybir.dt.float32, name="dsgB")]
            # t1 is written by the NEXT group's front phase while this
            # group's raw multiply still needs it -> double buffer by parity
            t1gs = [grp.tile([P, NG, C], mybir.dt.bfloat16, name="t1gA"),
                    grp.tile([P, NG, C], mybir.dt.bfloat16, name="t1gB")]
            idxgs = [grp.tile([P, NG, C], mybir.dt.int16, name=f"idx{i}")
                     for i in range(4)]
            rcbg = grp.tile([P, NG, C], mybir.dt.bfloat16)

            group_smalls = {}

            def load_group(g):
                lts = []
                for u in range(NG):
                    r0 = (g * NG + u) * P
                    lt = inp.tile([P, C], mybir.dt.float32)
                    # four quarter-row DMAs land on different queues,
                    # shortening the transfer the exp chain waits on
                    for q in range(4):
                        c0, c1 = q * (C // 4), (q + 1) * (C // 4)
                        nc.sync.dma_start(lt[:, c0:c1], l_in[r0:r0 + P, c0:c1])
                    lts.append(lt)
                return lts

            def fronts(g, lts=None):
                t1g = t1gs[g % 2]
                # wt already carries the sigmoid 0.5, so recipZ drives the
                # tanh directly and qZ = 0.25/Z only feeds the sqrt scale.
                smalls = []
                group_smalls[g] = smalls
                if lts is None:
                    lts = load_group(g)
                # pass 0: sort-padding zeros are DMA-independent - emit them
                # before the exps so ACT does them while the loads land.
                # Once only: the pads are the 24 smallest keys, so every sort
                # returns them to the tail positions as zeros.
                su = g % 2           # which half of the sort buffer
                if g < 2:
                    for u in range(NG):
                        nc.scalar.memzero(bufA[:, su * NG + u, C:N])
                # pass 1: exp + key-pack only - ACT streams pure exps and all
                # NG packs reach the DVE queue before the recip/matmul chain,
                # so the sort starts as early as possible
                es, Zs = [], []
                for u in range(NG):
                    e = front.tile([P, KEXT], mybir.dt.float32)
                    Z = smallp.tile([P, 1], mybir.dt.float32)
                    es.append(e)
                    Zs.append(Z)
                    nc.scalar.activation(out=e[:, 0:C], in_=lts[u][:],
                                         func=AF.Exp, accum_out=Z[:])
                    v = su * NG + u
                    _int_bitop(nc.vector, bufA[:, v, 0:C].bitcast(mybir.dt.int32),
                               e[:, 0:C].bitcast(mybir.dt.int32), -1024,
                               mybir.dt.int32, OP.bitwise_and,
                               in1=iota[:, 0:C], op1=OP.bitwise_or)
                # pass 2: normalization scalars + the fc matmul chain
                for u in range(NG):
                    e, Z = es[u], Zs[u]
                    nc.scalar.activation(out=e[:, C:C + 1], in_=Z[:],
                                         func=AF.Copy)
                    if g == 0:
                        # the front pool cycles the same NG buffers each
                        # group and nothing else writes the zero pad
                        nc.scalar.memzero(e[:, C + 1:KEXT])
                    recipZ = smallp.tile([P, 1], mybir.dt.float32)
                    qZ = smallp.tile([P, 1], mybir.dt.float32)
                    smalls.append(qZ)
                    nc.vector.reciprocal(out=recipZ[:], in_=Z[:])
                    nc.vector.tensor_scalar(out=qZ[:], in0=recipZ[:],
                                            scalar1=0.25, scalar2=None,
                                            op0=OP.mult)

                    ebT = ebtp.tile([P, 8, P], mybir.dt.bfloat16)
                    psT = psumF.tile([P, 8, P], mybir.dt.float32)
                    for k in range(8):
                        nc.tensor.transpose(psT[0:KCH, k, :],
                                            e[:, k * KCH:(k + 1) * KCH], ident[:])
                    nc.scalar.activation(out=ebT[0:KCH], in_=psT[0:KCH], func=AF.Copy)
                    psA = psumF.tile([P, 512], mybir.dt.float32)
                    psB = psumF.tile([P, C - 512], mybir.dt.float32)
                    for k in range(8):
                        nc.tensor.matmul(psA[:], ebT[0:KCH, k, :], wt[:, k, 0:512],
                                         start=(k == 0), stop=(k == 7))
                    for k in range(8):
                        nc.tensor.matmul(psB[:], ebT[0:KCH, k, :], wt[:, k, 512:C],
                                         start=(k == 0), stop=(k == 7))

                    nc.scalar.activation(out=t1g[:, u, 0:512], in_=psA[:],
                                         func=AF.Tanh, scale=recipZ[:])
                    nc.scalar.activation(out=t1g[:, u, 512:C - 1],
                                         in_=psB[:, 0:C - 1 - 512],
                                         func=AF.Tanh, scale=recipZ[:])
                    nc.scalar.activation(out=t1g[:, u, C - 1:C],
                                         in_=psB[:, C - 1 - 512:C - 512],
                                         func=AF.Copy, scale=recipZ[:], bias=-1.0)

            def epi1(g, spk):
                # sort-dependent, DVE only.  These must stay on the DVE: on
                # GPSIMD they execute while the DVE is still sorting the
                # next group and race with this group's epilogue.
                # The diffs subtract the packed keys directly: the sort
                # ordered them as positive floats, so spk[j] >= spk[j+1]
                # exactly (diff >= 0), and the index bits contribute noise
                # of the same 2^-13-relative scale as the truncation the
                # masked variant had.
                su = g % 2
                idxg, dsg = idxgs[g], dsgs[su]
                spk16 = spk[:].bitcast(mybir.dt.int16)
                spk16_even = bass.AP(tensor=spk16.tensor,
                                     offset=spk16.offset + su * NG * N * 2,
                                     ap=[spk16.ap[0], [2 * N, NG], [2, C]])
                _int_bitop(nc.vector, idxg[:], spk16_even, 1023,
                           mybir.dt.int16, OP.bitwise_and)
                so = spk.offset + su * NG * N
                nc.vector.scalar_tensor_tensor(
                    out=bass.AP(tensor=dsg.tensor, offset=dsg.offset,
                                ap=[dsg.ap[0], [C, NG], [1, C - 1]]),
                    in0=bass.AP(tensor=spk.tensor, offset=so,
                                ap=[spk.ap[0], [N, NG], [1, C - 1]]),
                    scalar=0.0,
                    in1=bass.AP(tensor=spk.tensor, offset=so + 1,
                                ap=[spk.ap[0], [N, NG], [1, C - 1]]),
                    op0=OP.bypass, op1=OP.subtract)

            def epi2(g):
                t1g = t1gs[g % 2]
                idxg, dsg = idxgs[g], dsgs[g % 2]
                for u in range(NG):
                    qZ = group_smalls[g][u]
                    nc.scalar.activation(out=dsg[:, u, 0:C - 1],
                                         in_=dsg[:, u, 0:C - 1],
                                         func=AF.Sqrt, scale=qZ[:])
                    # in_ must be a finite value: scale=0 still multiplies,
                    # and 0*NaN = NaN (cold SBUF can hold NaN patterns)
                    nc.scalar.activation(out=dsg[:, u, C - 1:C],
                                         in_=qZ[:],
                                         func=AF.Copy, scale=0.0, bias=1.0)
                nc.vector.scalar_tensor_tensor(out=dsg[:], in0=t1g[:],
                                               scalar=1.0, in1=dsg[:],
                                               op0=OP.add, op1=OP.mult)
                for u in range(NG):
                    # reverse cumsum as rc = raw @ L on the PE:
                    # transpose raw into k-chunks, then accumulate matmuls.
                    rawT = ebtp.tile([P, 8, P], mybir.dt.bfloat16, name="rawT")
                    psR = psumE.tile([P, 8, P], mybir.dt.float32)
                    for kc in range(8):
                        kw = min(KCH, C - kc * KCH)
                        nc.tensor.transpose(psR[0:kw, kc, :],
                                            dsg[:, u, kc * KCH:kc * KCH + kw],
                                            ident[:])
                    nc.scalar.activation(out=rawT[0:KCH, 0:7], in_=psR[0:KCH, 0:7],
                                         func=AF.Copy)
                    nc.scalar.activation(out=rawT[0:C - 7 * KCH, 7],
                                         in_=psR[0:C - 7 * KCH, 7], func=AF.Copy)
                    rcA = psumE.tile([P, 512], mybir.dt.float32)
                    rcB = psumE.tile([P, C - 512], mybir.dt.float32)
                    for kc in range(8):
                        kw = min(KCH, C - kc * KCH)
                        nc.tensor.matmul(rcA[:], rawT[0:kw, kc, :],
                                         L[0:kw, kc, 0:512],
                                         start=(kc == 0), stop=(kc == 7))
                    for kc in range(8):
                        kw = min(KCH, C - kc * KCH)
                        nc.tensor.matmul(rcB[:], rawT[0:kw, kc, :],
                                         L[0:kw, kc, 512:C],
                                         start=(kc == 0), stop=(kc == 7))
                    nc.scalar.activation(out=rcbg[:, u, 0:512], in_=rcA[:],
                                         func=AF.Copy)
                    nc.scalar.activation(out=rcbg[:, u, 512:C], in_=rcB[:],
                                         func=AF.Copy)
                    # rc is already bf16-quantized (the scatter moves 2-byte
                    # data), so DMA it out as bf16 and upcast on the host.
                    fitb = outp.tile([P, N], mybir.dt.bfloat16)
                    nc.gpsimd.local_scatter(out_ap=fitb[:], data_ap=rcbg[:, u, :],
                                            idxs_ap=idxg[:, u, :], channels=P,
                                            num_elems=N, num_idxs=C)
                    r0 = (g * NG + u) * P
                    nc.sync.dma_start(out_d[r0:r0 + P, :], fitb[:, 0:C])

            # 8-4-4 sort schedule: one 8-tile sort amortizes the per-substage
            # fixed cost over the first half; the second half runs as two
            # 4-tile sorts so the post-last-sort drain (epi chain + GPSIMD
            # scatters) covers 4 tiles instead of 8.
            lts0 = load_group(0)
            deferred_init()
            fronts(0, lts0)                        # -> half 0
            fronts(1)                              # -> half 1
            spk = _emit_sort(nc, bufA, 0, NSG)     # sort A: tiles 0-7
            epi1(0, spk)
            epi1(1, spk)
            fronts(2)                              # -> half 0 (after epi1(0))
            spk2 = _emit_sort(nc, bufA, 0, NG)     # sort B: tiles 8-11
            epi2(0)
            epi2(1)
            epi1(2, spk2)
            fronts(3)                              # -> half 1 (after epi1(1))
            spk3 = _emit_sort(nc, bufA, NG, NG)    # sort C: tiles 12-15
            epi2(2)
            epi1(3, spk3)
            epi2(3)
    # our CE instructions are plain COPY at the ISA level; record the op so
    # dve_table_for_ops hijacks COPY's opcode row in the per-NEFF table
    nc.m.ant_custom_dve_ops = sorted({*nc.m.ant_custom_dve_ops, MINMAX_NAME})
    nc.compile()
    return nc


def _prep_wt(W, b):
    wt_ext = np.zeros((KEXT, C), dtype=np.float32)
    wt_ext[:C, :] = W.T
    wt_ext[C, :] = b
    # tanh(fc * 0.5/Z) is computed as tanh(fc_half / Z) with the 0.5
    # pre-folded here (exact: exponent shift); column 999 skips the
    # sigmoid in the reference, so it keeps full scale.
    wt_ext[:, 0:C - 1] *= 0.5
    return wt_ext.astype(ml_dtypes.bfloat16)


def _prep_ltri():
    """L[p, kc, j] = 1 if kc*KCH + p >= j: the k-chunked lower-triangular
    ones matrix whose matmul gives the inclusive cumsum."""
    k_idx = (np.arange(8)[None, :] * KCH + np.arange(KCH)[:, None])
    ltri = (k_idx[:, :, None] >= np.arange(C)[None, None, :])
    return ltri.astype(ml_dtypes.bfloat16)


def kernel(logits, W, b):
    logits = np.ascontiguousarray(np.asarray(logits, dtype=np.float32))
    W = np.asarray(W, dtype=np.float32)
    b = np.asarray(b, dtype=np.float32)
    assert logits.shape == (BATCH, C)
    wt_ext = _prep_wt(W, b)
    ltri = _prep_ltri()

    nc = build_nc()
    in_maps = [
        {"logits": logits[i * ROWS:(i + 1) * ROWS], "wt": wt_ext, "ltri": ltri}
        for i in range(NCORES)
    ]
    res = run_bass_kernel_spmd(nc, in_maps, core_ids=list(range(NCORES)))
    out = np.concatenate(
        [np.asarray(res.results[i]["out"]) for i in range(NCORES)], axis=0)
    return out.astype(np.float32)

